# revision 22
# baseline (speedup 1.0000x reference)
"""Energy refinement kernel for Trainium2 (8 NeuronCores, SPMD row-sharded).

Math notes
----------
reference() computes, for L=4096 coords [L,3] and a 0/1 contact_map [L,L]:
  e_bond  = mean((||c[i+1]-c[i]|| - 6)^2)                       (O(L), host)
  d[i,j]  = ||c_i - c_j|| (+1e-8)
  e_clash = sum_{j>=i+3} relu(3.4-d)^2 / L
  e_pair  = sum_{contact & |i-j|>=3} (d-9)^2 / max(n_contacts,1)
  total   = e_bond + 2*e_clash + 0.5*e_pair

Device strategy (row-sharded over 8 cores, 512 rows each):
  d2 = A @ B^T with A=[c,|c|^2,1], B=[-2c,1,|c|^2]  (K=5 matmul -> PSUM)
  d  = sqrt(max(d2, 0))   (clamp kills fp32 matmul round-off on the diagonal)
  Each core returns UNMASKED per-partition partial sums over its block:
    clash part: sum (min(d,3.4)-3.4)^2     == sum relu(3.4-d)^2
    pair  part: sum contact*(d-9)^2
    n     part: column sums of contact     (PE ones-matmul, PSUM-accumulated)
Host finishing (float64): the separation masks only remove the 5-diagonal
band |i-j|<=2, which is O(L) work: subtract a band correction computed with
the device's own formula, halve the clash sum (d is symmetric; reference
sums the upper triangle only), divide, and add the bond term.
"""

import numpy as np

L = 4096
NCORES = 8
RPC = L // NCORES          # 512 rows per core
RT = RPC // 128            # 4 row tiles of 128 partitions
CMW = 2048                 # col macro width
CM = L // CMW              # 2 col macros
NSLOT = RT * CM            # 8 accumulator slots per core
MIN_DIST = 3.4
TARGET_DIST = 9.0
IDEAL_BOND = 6.0
W_BOND, W_CLASH, W_PAIR = 1.0, 2.0, 0.5


def _build_nc():
    import concourse.bass as bass
    import concourse.bacc as bacc
    import concourse.mybir as mybir
    import concourse.tile as tile

    f32 = mybir.dt.float32
    AF = mybir.ActivationFunctionType
    ALU = mybir.AluOpType

    # Bacc (not Bass): its compile() runs move_matmul_waits_to_ldweights,
    # required because walrus allows only one sync wait per Matmult.
    nc = bacc.Bacc(None)
    # ab = [at | bt] concatenated so a single DMA (one wait semaphore) loads
    # both matmul operands — walrus allows only one sync wait per Matmult.
    ab = nc.declare_dram_parameter("ab", [5, RPC + L], f32, isOutput=False)
    cmap = nc.declare_dram_parameter("cmap", [RPC, L], f32, isOutput=False)
    o_clash = nc.declare_dram_parameter("o_clash", [128, NSLOT], f32, isOutput=True)
    o_pair = nc.declare_dram_parameter("o_pair", [128, NSLOT], f32, isOutput=True)
    o_n = nc.declare_dram_parameter("o_n", [1, 512], f32, isOutput=True)

    HMW = CMW // 2  # 1024: psum half-macro width (2 banks)
    n_pn = RT * CM * (CMW // 512)

    with tile.TileContext(nc) as tc:
        with (
            tc.tile_pool(name="const", bufs=1) as constp,
            tc.tile_pool(name="cmapp", bufs=3) as cmapp,
            tc.tile_pool(name="work", bufs=2) as work,
            tc.tile_pool(name="accp", bufs=1) as accp,
            tc.tile_pool(name="psum", bufs=3, space=bass.MemorySpace.PSUM) as psum,
            tc.tile_pool(name="psumn", bufs=1, space=bass.MemorySpace.PSUM) as psumn,
        ):
            ab_sb = constp.tile([5, RPC + L], f32)
            ones_sb = constp.tile([128, 1], f32)
            nc.sync.dma_start(ab_sb[:], ab[:])
            nc.gpsimd.memset(ones_sb[:], 1.0)
            at_sb = ab_sb[:, :RPC]
            bt_sb = ab_sb[:, RPC:]

            acc_clash = accp.tile([128, NSLOT], f32)
            acc_pair = accp.tile([128, NSLOT], f32)
            pn = psumn.tile([1, 512], f32)

            mm = 0
            for it in range(RT):
                for jm in range(CM):
                    slot = it * CM + jm
                    ct = cmapp.tile([128, CMW], f32, tag="ct")
                    nc.sync.dma_start(
                        ct[:],
                        cmap[it * 128 : (it + 1) * 128, jm * CMW : (jm + 1) * CMW],
                    )
                    lhs = at_sb[:, it * 128 : (it + 1) * 128]
                    halves = []
                    for h in range(2):
                        ps = psum.tile([128, HMW], f32, tag="d2")
                        base = jm * CMW + h * HMW
                        for q in range(HMW // 512):
                            nc.tensor.matmul(
                                ps[:, q * 512 : (q + 1) * 512],
                                lhs,
                                bt_sb[:, base + q * 512 : base + (q + 1) * 512],
                                start=True,
                                stop=True,
                            )
                        halves.append(ps)
                    # contact count: PSUM-accumulated column sums on the PE
                    for q in range(CMW // 512):
                        nc.tensor.matmul(
                            pn[:],
                            ones_sb[:],
                            ct[:, q * 512 : (q + 1) * 512],
                            start=(mm == 0),
                            stop=(mm == n_pn - 1),
                        )
                        mm += 1

                    # clamp fp32 matmul round-off (diagonal) to 0 before sqrt
                    t_u = work.tile([128, CMW], f32, tag="t_u")
                    for h in range(2):
                        nc.scalar.activation(
                            t_u[:, h * HMW : (h + 1) * HMW], halves[h][:], AF.Relu
                        )
                    t_d = work.tile([128, CMW], f32, tag="t_d")
                    nc.scalar.activation(t_d[:], t_u[:], AF.Sqrt)

                    # clash: sum (min(d,3.4)-3.4)^2 — DVE min/sub, ACT square+accum
                    t_m = work.tile([128, CMW], f32, tag="t_m")
                    nc.vector.tensor_scalar(
                        t_m[:], t_d[:], MIN_DIST, MIN_DIST, ALU.min, ALU.subtract
                    )
                    t_j0 = work.tile([128, CMW], f32, tag="junk")
                    nc.scalar.activation(
                        t_j0[:],
                        t_m[:],
                        AF.Square,
                        accum_out=acc_clash[:, slot : slot + 1],
                    )

                    # pair: sum contact*(d-9)^2 = sum ((d-9)*contact)^2 since
                    # contact is 0/1 — two fused DVE scalar_tensor_tensor passes
                    t_x = work.tile([128, CMW], f32, tag="t_x")
                    nc.vector.scalar_tensor_tensor(
                        t_x[:], t_d[:], TARGET_DIST, ct[:], ALU.subtract, ALU.mult
                    )
                    t_j1 = work.tile([128, CMW], f32, tag="junk")
                    nc.vector.scalar_tensor_tensor(
                        t_j1[:],
                        t_x[:],
                        1.0,
                        t_x[:],
                        ALU.mult,
                        ALU.mult,
                        accum_out=acc_pair[:, slot : slot + 1],
                    )

            pn_sb = accp.tile([1, 512], f32)
            nc.vector.tensor_copy(pn_sb[:], pn[:])
            nc.sync.dma_start(o_clash[:], acc_clash[:])
            nc.sync.dma_start(o_pair[:], acc_pair[:])
            nc.sync.dma_start(o_n[:], pn_sb[:])
    nc.compile()
    return nc


def _augmented(coords):
    """A, B in float32 such that (A @ B.T)[i,j] ~= ||c_i - c_j||^2."""
    c = np.asarray(coords, dtype=np.float32)
    n2 = (c * c).sum(axis=1, dtype=np.float32).astype(np.float32)
    one = np.ones((c.shape[0], 1), dtype=np.float32)
    A = np.concatenate([c, n2[:, None], one], axis=1).astype(np.float32)
    B = np.concatenate([(-2.0 * c).astype(np.float32), one, n2[:, None]], axis=1)
    return A, B.astype(np.float32)


def _band_correction(A, B, contact_map):
    """Sum of device-formula clash^2 / contact*(d-9)^2 over |i-j| <= 2 (f64)."""
    Lc = A.shape[0]
    band_clash = 0.0
    band_pair = 0.0
    for k in range(-2, 3):
        i = np.arange(max(0, -k), min(Lc, Lc - k))
        j = i + k
        # emulate the PE's sequential fp32 K-dim accumulation
        s = np.zeros(len(i), dtype=np.float32)
        for m in range(5):
            s = (s + A[i, m] * B[j, m]).astype(np.float32)
        d = np.sqrt(np.maximum(s, np.float32(0.0)).astype(np.float64))
        clash = np.minimum(d, MIN_DIST) - MIN_DIST
        band_clash += float((clash * clash).sum())
        cm = contact_map[i, j].astype(np.float64)
        band_pair += float((cm * (d - TARGET_DIST) ** 2).sum())
    return band_clash, band_pair


_CACHE = {}


def kernel(coords, contact_map):
    from concourse.bass_utils import run_bass_kernel_spmd

    coords = np.asarray(coords, dtype=np.float32)
    contact_map = np.ascontiguousarray(np.asarray(contact_map, dtype=np.float32))
    A, B = _augmented(coords)
    AT = np.ascontiguousarray(A.T)  # [5, L]
    BT = np.ascontiguousarray(B.T)  # [5, L]

    in_maps = []
    for r in range(NCORES):
        ab_r = np.concatenate([AT[:, r * RPC : (r + 1) * RPC], BT], axis=1)
        in_maps.append(
            {
                "ab": np.ascontiguousarray(ab_r),
                "cmap": np.ascontiguousarray(contact_map[r * RPC : (r + 1) * RPC]),
            }
        )

    if "nc" not in _CACHE:
        _CACHE["nc"] = _build_nc()
    res = run_bass_kernel_spmd(_CACHE["nc"], in_maps, list(range(NCORES))).results

    S_clash = 0.0
    S_pair = 0.0
    S_n = 0.0
    for r in range(NCORES):
        S_clash += float(res[r]["o_clash"].astype(np.float64).sum())
        S_pair += float(res[r]["o_pair"].astype(np.float64).sum())
        S_n += float(res[r]["o_n"].astype(np.float64).sum())

    band_clash, band_pair = _band_correction(A, B, contact_map)
    n_pairs = max(round(S_n), 1)

    e_clash = (S_clash - band_clash) / 2.0 / L
    e_pair = (S_pair - band_pair) / n_pairs

    diff = coords.astype(np.float64)[1:] - coords.astype(np.float64)[:-1]
    bond = np.sqrt((diff * diff).sum(axis=1))
    e_bond = float(((bond - IDEAL_BOND) ** 2).mean())

    total = W_BOND * e_bond + W_CLASH * e_clash + W_PAIR * e_pair
    return np.array([total], dtype=np.float32)


# revision 27
# speedup vs baseline: 12.2597x; 12.2597x over previous
"""Energy refinement kernel for Trainium2 (8 NeuronCores, SPMD row-sharded).

Math notes
----------
reference() computes, for L=4096 coords [L,3] and a 0/1 contact_map [L,L]:
  e_bond  = mean((||c[i+1]-c[i]|| - 6)^2)                       (O(L), host)
  d[i,j]  = ||c_i - c_j|| (+1e-8)
  e_clash = sum_{j>=i+3} relu(3.4-d)^2 / L
  e_pair  = sum_{contact & |i-j|>=3} (d-9)^2 / max(n_contacts,1)
  total   = e_bond + 2*e_clash + 0.5*e_pair

Device strategy (folded symmetry, row-sharded over 8 cores):
  d2 = A @ B^T with A=[c,|c|^2,1], B=[-2c,1,|c|^2]  (K=5 matmul -> PSUM)
  d  = sqrt(max(d2, 0))   (clamp kills fp32 matmul round-off on the diagonal)
  d is symmetric, so each unordered pair is visited once: a 128-row block a
  only processes the 17-block cyclic column span [128a, 128a+2176) — block
  offsets 0..16 of 32.  Offset-0 (within-block) and offset-16 pairs appear
  twice across the whole grid and are halved.  Per 128-row tile the device
  returns per-partition sums of:
    clash = (min(d,3.4)-3.4)^2 = relu(3.4-d)^2, in 3 column sub-ranges
            (diag block / offsets 1..15 / offset-16 block) so the host can
            halve the double-counted slots;
    pair  = ((d-9)*cfold)^2 where cfold = sqrt(w*(c_ij+c_ji)) is built on
            the host (w = 1/2 on the offset-0/16 blocks) — squaring on
            device turns it into w*(c_ij+c_ji)*(d-9)^2.
  The column span and cfold differ per core, so both are shipped as data
  (pre-gathered B columns) keeping the SPMD program identical on all cores.
Host finishing (float64, all O(L) or one O(L^2) pass for cfold):
  subtract the i=j terms and the |i-j|<=2 band (computed with the device's
  own f32 formula), halve the double-counted slots, divide, add bond term.
"""

import numpy as np

L = 4096
NCORES = 8
RPC = L // NCORES          # 512 rows per core
RT = RPC // 128            # 4 row tiles of 128 partitions
SPAN = 17 * 128            # 2176 columns per row tile (block offsets 0..16)
MIN_DIST = 3.4
TARGET_DIST = 9.0
IDEAL_BOND = 6.0
W_BOND, W_CLASH, W_PAIR = 1.0, 2.0, 0.5


def _build_nc(reps=1):
    import concourse.bass as bass
    import concourse.bacc as bacc
    import concourse.mybir as mybir
    import concourse.tile as tile

    f32 = mybir.dt.float32
    AF = mybir.ActivationFunctionType
    ALU = mybir.AluOpType

    # Bacc (not Bass): its compile() runs move_matmul_waits_to_ldweights,
    # required because walrus allows only one sync wait per Matmult.
    nc = bacc.Bacc(None)
    # ab = [at | btfold x4] so a single DMA (one wait semaphore) loads all
    # matmul operands — walrus allows only one sync wait per Matmult.
    ab = nc.declare_dram_parameter("ab", [5, RPC + RT * SPAN], f32, isOutput=False)
    cfold = nc.declare_dram_parameter("cfold", [RPC, SPAN], f32, isOutput=False)
    o_clash = nc.declare_dram_parameter("o_clash", [128, RT * 3], f32, isOutput=True)
    o_pair = nc.declare_dram_parameter("o_pair", [128, RT], f32, isOutput=True)

    HSP = SPAN // 2  # 1088: clamp op width

    with tile.TileContext(nc) as tc:
        with (
            tc.tile_pool(name="const", bufs=1) as constp,
            tc.tile_pool(name="cfp", bufs=3) as cfp,
            tc.tile_pool(name="work", bufs=2) as work,
            tc.tile_pool(name="accp", bufs=1) as accp,
            tc.tile_pool(name="psum", bufs=1, space=bass.MemorySpace.PSUM) as psum,
        ):
            ab_sb = constp.tile([5, RPC + RT * SPAN], f32)
            nc.sync.dma_start(ab_sb[:], ab[:])

            acc_clash = accp.tile([128, RT * 3], f32)
            acc_pair = accp.tile([128, RT], f32)

            for rep in range(reps):
                for it in range(RT):
                    ct = cfp.tile([128, SPAN], f32, tag="ct")
                    nc.sync.dma_start(
                        ct[:], cfold[it * 128 : (it + 1) * 128, :]
                    )
                    lhs = ab_sb[:, it * 128 : (it + 1) * 128]
                    rbase = RPC + it * SPAN
                    ps = psum.tile([128, SPAN], f32, tag="d2")
                    off = 0
                    for n in (512, 512, 512, 512, 128):
                        nc.tensor.matmul(
                            ps[:, off : off + n],
                            lhs,
                            ab_sb[:, rbase + off : rbase + off + n],
                            start=True,
                            stop=True,
                        )
                        off += n

                    # clamp fp32 matmul round-off (diagonal) to 0 before sqrt
                    t_u = work.tile([128, SPAN], f32, tag="t_u")
                    nc.scalar.activation(t_u[:], ps[:], AF.Relu)
                    t_d = work.tile([128, SPAN], f32, tag="t_d")
                    nc.scalar.activation(t_d[:], t_u[:], AF.Sqrt)

                    # clash: relu(3.4-d)^2 summed per column sub-range
                    t_m = work.tile([128, SPAN], f32, tag="t_m")
                    nc.vector.tensor_scalar(
                        t_m[:], t_d[:], MIN_DIST, MIN_DIST, ALU.min, ALU.subtract
                    )
                    t_j0 = work.tile([128, SPAN], f32, tag="junk")
                    for lo, hi, s in ((0, 128, 0), (128, 2048, 1), (2048, SPAN, 2)):
                        nc.scalar.activation(
                            t_j0[:, lo:hi],
                            t_m[:, lo:hi],
                            AF.Square,
                            accum_out=acc_clash[:, it * 3 + s : it * 3 + s + 1],
                        )

                    # pair: ((d-9)*cfold)^2 summed — two fused DVE passes
                    t_x = work.tile([128, SPAN], f32, tag="t_x")
                    nc.vector.scalar_tensor_tensor(
                        t_x[:], t_d[:], TARGET_DIST, ct[:], ALU.subtract, ALU.mult
                    )
                    t_j1 = work.tile([128, SPAN], f32, tag="junk")
                    nc.vector.scalar_tensor_tensor(
                        t_j1[:],
                        t_x[:],
                        1.0,
                        t_x[:],
                        ALU.mult,
                        ALU.mult,
                        accum_out=acc_pair[:, it : it + 1],
                    )

            nc.sync.dma_start(o_clash[:], acc_clash[:])
            nc.sync.dma_start(o_pair[:], acc_pair[:])
    nc.compile()
    return nc


def _augmented(coords):
    """A, B in float32 such that (A @ B.T)[i,j] ~= ||c_i - c_j||^2."""
    c = np.asarray(coords, dtype=np.float32)
    n2 = (c * c).sum(axis=1, dtype=np.float32).astype(np.float32)
    one = np.ones((c.shape[0], 1), dtype=np.float32)
    A = np.concatenate([c, n2[:, None], one], axis=1).astype(np.float32)
    B = np.concatenate([(-2.0 * c).astype(np.float32), one, n2[:, None]], axis=1)
    return A, B.astype(np.float32)


def _emul_offset(A, B, offset):
    """f32-emulated device d-hat for pairs (i, (i+offset) % L)."""
    i = np.arange(L)
    j = (i + offset) % L
    s = np.zeros(L, dtype=np.float32)
    for m in range(5):
        s = (s + A[i, m] * B[j, m]).astype(np.float32)
    return np.sqrt(np.maximum(s, np.float32(0.0)).astype(np.float64)), i, j


def _host_inputs(coords, contact_map):
    A, B = _augmented(coords)
    AT = np.ascontiguousarray(A.T)  # [5, L]
    BT = np.ascontiguousarray(B.T)
    w = np.ones(SPAN, dtype=np.float32)
    w[:128] = 0.5
    w[2048:] = 0.5
    in_maps = []
    for r in range(NCORES):
        parts = [AT[:, r * RPC : (r + 1) * RPC]]
        cf_r = np.empty((RPC, SPAN), dtype=np.float32)
        for it in range(RT):
            a = r * RT + it
            i0 = a * 128
            cols = np.arange(i0, i0 + SPAN) % L
            parts.append(BT[:, cols])
            cf = (
                contact_map[i0 : i0 + 128][:, cols]
                + contact_map[cols][:, i0 : i0 + 128].T
            ) * w
            np.sqrt(cf, out=cf)
            cf_r[it * 128 : (it + 1) * 128] = cf
        in_maps.append(
            {
                "ab": np.ascontiguousarray(np.concatenate(parts, axis=1)),
                "cfold": cf_r,
            }
        )
    return A, B, in_maps


_CACHE = {}


def kernel(coords, contact_map):
    from concourse.bass_utils import run_bass_kernel_spmd

    coords = np.asarray(coords, dtype=np.float32)
    contact_map = np.ascontiguousarray(np.asarray(contact_map, dtype=np.float32))
    A, B, in_maps = _host_inputs(coords, contact_map)

    if "nc" not in _CACHE:
        _CACHE["nc"] = _build_nc()
    res = run_bass_kernel_spmd(_CACHE["nc"], in_maps, list(range(NCORES))).results

    S_main = S_diag = S_s16 = S_pair = 0.0
    for r in range(NCORES):
        oc = res[r]["o_clash"].astype(np.float64)
        S_diag += float(oc[:, 0::3].sum())
        S_main += float(oc[:, 1::3].sum())
        S_s16 += float(oc[:, 2::3].sum())
        S_pair += float(res[r]["o_pair"].astype(np.float64).sum())

    # host corrections, all O(L), emulating the device's own f32 formula
    dh0, ii, _ = _emul_offset(A, B, 0)
    cl0 = np.minimum(dh0, MIN_DIST) - MIN_DIST
    emul_ii = float((cl0 * cl0).sum())
    pair_ii = float(
        (contact_map[ii, ii].astype(np.float64) * (dh0 - TARGET_DIST) ** 2).sum()
    )
    band_clash = 0.0
    band_pair = 0.0
    for s_off in (1, 2):
        dh, i, j = _emul_offset(A, B, s_off)
        keep = i + s_off < L  # wrapped pairs have linear sep >= 3: not banded
        cl = np.minimum(dh[keep], MIN_DIST) - MIN_DIST
        band_clash += float((cl * cl).sum())
        cw = (
            contact_map[i[keep], j[keep]].astype(np.float64)
            + contact_map[j[keep], i[keep]].astype(np.float64)
        )
        band_pair += float((cw * (dh[keep] - TARGET_DIST) ** 2).sum())

    S_clash_u = S_main + (S_diag - emul_ii) / 2.0 + S_s16 / 2.0
    e_clash = (S_clash_u - band_clash) / L

    n_pairs = max(int(round(float(contact_map.sum(dtype=np.float64)))), 1)
    e_pair = (S_pair - pair_ii - band_pair) / n_pairs

    diff = coords.astype(np.float64)[1:] - coords.astype(np.float64)[:-1]
    bond = np.sqrt((diff * diff).sum(axis=1))
    e_bond = float(((bond - IDEAL_BOND) ** 2).mean())

    total = W_BOND * e_bond + W_CLASH * e_clash + W_PAIR * e_pair
    return np.array([total], dtype=np.float32)


# revision 28
# speedup vs baseline: 13.4516x; 1.0972x over previous
"""Energy refinement kernel for Trainium2 (8 NeuronCores, SPMD row-sharded).

Math notes
----------
reference() computes, for L=4096 coords [L,3] and a 0/1 contact_map [L,L]:
  e_bond  = mean((||c[i+1]-c[i]|| - 6)^2)                       (O(L), host)
  d[i,j]  = ||c_i - c_j|| (+1e-8)
  e_clash = sum_{j>=i+3} relu(3.4-d)^2 / L
  e_pair  = sum_{contact & |i-j|>=3} (d-9)^2 / max(n_contacts,1)
  total   = e_bond + 2*e_clash + 0.5*e_pair

Device strategy (folded symmetry, row-sharded over 8 cores):
  d2 = A @ B^T with A=[c,|c|^2,1], B=[-2c,1,|c|^2]  (K=5 matmul -> PSUM)
  d  = sqrt(max(d2, 0))   (clamp kills fp32 matmul round-off on the diagonal)
  d is symmetric, so each unordered pair is visited once: a 128-row block a
  only processes the 17-block cyclic column span [128a, 128a+2176) — block
  offsets 0..16 of 32.  Offset-0 (within-block) and offset-16 pairs appear
  twice across the whole grid and are halved.  Per 128-row tile the device
  returns per-partition sums of:
    clash = (min(d,3.4)-3.4)^2 = relu(3.4-d)^2, in 3 column sub-ranges
            (diag block / offsets 1..15 / offset-16 block) so the host can
            halve the double-counted slots;
    pair  = ((d-9)*cfold)^2 where cfold = sqrt(w*(c_ij+c_ji)) is built on
            the host (w = 1/2 on the offset-0/16 blocks) — squaring on
            device turns it into w*(c_ij+c_ji)*(d-9)^2.
  The column span and cfold differ per core, so both are shipped as data
  (pre-gathered B columns) keeping the SPMD program identical on all cores.
Host finishing (float64, all O(L) or one O(L^2) pass for cfold):
  subtract the i=j terms and the |i-j|<=2 band (computed with the device's
  own f32 formula), halve the double-counted slots, divide, add bond term.
"""

import numpy as np

L = 4096
NCORES = 8
RPC = L // NCORES          # 512 rows per core
RT = RPC // 128            # 4 row tiles of 128 partitions
SPAN = 17 * 128            # 2176 columns per row tile (block offsets 0..16)
MIN_DIST = 3.4
TARGET_DIST = 9.0
IDEAL_BOND = 6.0
W_BOND, W_CLASH, W_PAIR = 1.0, 2.0, 0.5


def _build_nc(reps=1):
    import concourse.bass as bass
    import concourse.bacc as bacc
    import concourse.mybir as mybir
    import concourse.tile as tile

    f32 = mybir.dt.float32
    AF = mybir.ActivationFunctionType
    ALU = mybir.AluOpType

    # Bacc (not Bass): its compile() runs move_matmul_waits_to_ldweights,
    # required because walrus allows only one sync wait per Matmult.
    nc = bacc.Bacc(None)
    # ab = [at | btfold x4] so a single DMA (one wait semaphore) loads all
    # matmul operands — walrus allows only one sync wait per Matmult.
    ab = nc.declare_dram_parameter("ab", [5, RPC + RT * SPAN], f32, isOutput=False)
    cfold = nc.declare_dram_parameter("cfold", [RPC, SPAN], f32, isOutput=False)
    o_clash = nc.declare_dram_parameter("o_clash", [128, RT * 3], f32, isOutput=True)
    o_pair = nc.declare_dram_parameter("o_pair", [128, RT], f32, isOutput=True)

    HSP = SPAN // 2  # 1088: clamp op width

    with tile.TileContext(nc) as tc:
        with (
            tc.tile_pool(name="const", bufs=1) as constp,
            tc.tile_pool(name="cfp", bufs=3) as cfp,
            tc.tile_pool(name="work", bufs=2) as work,
            tc.tile_pool(name="accp", bufs=1) as accp,
            tc.tile_pool(name="psum", bufs=1, space=bass.MemorySpace.PSUM) as psum,
        ):
            ab_sb = constp.tile([5, RPC + RT * SPAN], f32)
            nc.sync.dma_start(ab_sb[:], ab[:])

            acc_clash = accp.tile([128, RT * 3], f32)
            acc_pair = accp.tile([128, RT], f32)

            for rep in range(reps):
                for it in range(RT):
                    ct = cfp.tile([128, SPAN], f32, tag="ct")
                    nc.sync.dma_start(
                        ct[:], cfold[it * 128 : (it + 1) * 128, :]
                    )
                    lhs = ab_sb[:, it * 128 : (it + 1) * 128]
                    rbase = RPC + it * SPAN
                    ps = psum.tile([128, SPAN], f32, tag="d2")
                    off = 0
                    for n in (512, 512, 512, 512, 128):
                        nc.tensor.matmul(
                            ps[:, off : off + n],
                            lhs,
                            ab_sb[:, rbase + off : rbase + off + n],
                            start=True,
                            stop=True,
                        )
                        off += n

                    # clamp fp32 matmul round-off (diagonal) to 0 before sqrt
                    t_u = work.tile([128, SPAN], f32, tag="t_u")
                    for h in range(2):
                        nc.scalar.activation(
                            t_u[:, h * HSP : (h + 1) * HSP],
                            ps[:, h * HSP : (h + 1) * HSP],
                            AF.Relu,
                        )
                    t_d = work.tile([128, SPAN], f32, tag="t_d")
                    nc.scalar.activation(t_d[:], t_u[:], AF.Sqrt)

                    # clash: relu(3.4-d)^2 summed per column sub-range
                    t_m = work.tile([128, SPAN], f32, tag="t_m")
                    nc.vector.tensor_scalar(
                        t_m[:], t_d[:], MIN_DIST, MIN_DIST, ALU.min, ALU.subtract
                    )
                    t_j0 = work.tile([128, SPAN], f32, tag="junk")
                    for lo, hi, s in ((0, 128, 0), (128, 2048, 1), (2048, SPAN, 2)):
                        nc.scalar.activation(
                            t_j0[:, lo:hi],
                            t_m[:, lo:hi],
                            AF.Square,
                            accum_out=acc_clash[:, it * 3 + s : it * 3 + s + 1],
                        )

                    # pair: ((d-9)*cfold)^2 summed — two fused DVE passes
                    t_x = work.tile([128, SPAN], f32, tag="t_x")
                    nc.vector.scalar_tensor_tensor(
                        t_x[:], t_d[:], TARGET_DIST, ct[:], ALU.subtract, ALU.mult
                    )
                    t_j1 = work.tile([128, SPAN], f32, tag="junk")
                    nc.vector.scalar_tensor_tensor(
                        t_j1[:],
                        t_x[:],
                        1.0,
                        t_x[:],
                        ALU.mult,
                        ALU.mult,
                        accum_out=acc_pair[:, it : it + 1],
                    )

            nc.sync.dma_start(o_clash[:], acc_clash[:])
            nc.sync.dma_start(o_pair[:], acc_pair[:])
    nc.compile()
    return nc


def _augmented(coords):
    """A, B in float32 such that (A @ B.T)[i,j] ~= ||c_i - c_j||^2."""
    c = np.asarray(coords, dtype=np.float32)
    n2 = (c * c).sum(axis=1, dtype=np.float32).astype(np.float32)
    one = np.ones((c.shape[0], 1), dtype=np.float32)
    A = np.concatenate([c, n2[:, None], one], axis=1).astype(np.float32)
    B = np.concatenate([(-2.0 * c).astype(np.float32), one, n2[:, None]], axis=1)
    return A, B.astype(np.float32)


def _emul_offset(A, B, offset):
    """f32-emulated device d-hat for pairs (i, (i+offset) % L)."""
    i = np.arange(L)
    j = (i + offset) % L
    s = np.zeros(L, dtype=np.float32)
    for m in range(5):
        s = (s + A[i, m] * B[j, m]).astype(np.float32)
    return np.sqrt(np.maximum(s, np.float32(0.0)).astype(np.float64)), i, j


def _host_inputs(coords, contact_map):
    A, B = _augmented(coords)
    AT = np.ascontiguousarray(A.T)  # [5, L]
    BT = np.ascontiguousarray(B.T)
    w = np.ones(SPAN, dtype=np.float32)
    w[:128] = 0.5
    w[2048:] = 0.5
    in_maps = []
    for r in range(NCORES):
        parts = [AT[:, r * RPC : (r + 1) * RPC]]
        cf_r = np.empty((RPC, SPAN), dtype=np.float32)
        for it in range(RT):
            a = r * RT + it
            i0 = a * 128
            cols = np.arange(i0, i0 + SPAN) % L
            parts.append(BT[:, cols])
            cf = (
                contact_map[i0 : i0 + 128][:, cols]
                + contact_map[cols][:, i0 : i0 + 128].T
            ) * w
            np.sqrt(cf, out=cf)
            cf_r[it * 128 : (it + 1) * 128] = cf
        in_maps.append(
            {
                "ab": np.ascontiguousarray(np.concatenate(parts, axis=1)),
                "cfold": cf_r,
            }
        )
    return A, B, in_maps


_CACHE = {}


def kernel(coords, contact_map):
    from concourse.bass_utils import run_bass_kernel_spmd

    coords = np.asarray(coords, dtype=np.float32)
    contact_map = np.ascontiguousarray(np.asarray(contact_map, dtype=np.float32))
    A, B, in_maps = _host_inputs(coords, contact_map)

    if "nc" not in _CACHE:
        _CACHE["nc"] = _build_nc()
    res = run_bass_kernel_spmd(_CACHE["nc"], in_maps, list(range(NCORES))).results

    S_main = S_diag = S_s16 = S_pair = 0.0
    for r in range(NCORES):
        oc = res[r]["o_clash"].astype(np.float64)
        S_diag += float(oc[:, 0::3].sum())
        S_main += float(oc[:, 1::3].sum())
        S_s16 += float(oc[:, 2::3].sum())
        S_pair += float(res[r]["o_pair"].astype(np.float64).sum())

    # host corrections, all O(L), emulating the device's own f32 formula
    dh0, ii, _ = _emul_offset(A, B, 0)
    cl0 = np.minimum(dh0, MIN_DIST) - MIN_DIST
    emul_ii = float((cl0 * cl0).sum())
    pair_ii = float(
        (contact_map[ii, ii].astype(np.float64) * (dh0 - TARGET_DIST) ** 2).sum()
    )
    band_clash = 0.0
    band_pair = 0.0
    for s_off in (1, 2):
        dh, i, j = _emul_offset(A, B, s_off)
        keep = i + s_off < L  # wrapped pairs have linear sep >= 3: not banded
        cl = np.minimum(dh[keep], MIN_DIST) - MIN_DIST
        band_clash += float((cl * cl).sum())
        cw = (
            contact_map[i[keep], j[keep]].astype(np.float64)
            + contact_map[j[keep], i[keep]].astype(np.float64)
        )
        band_pair += float((cw * (dh[keep] - TARGET_DIST) ** 2).sum())

    S_clash_u = S_main + (S_diag - emul_ii) / 2.0 + S_s16 / 2.0
    e_clash = (S_clash_u - band_clash) / L

    n_pairs = max(int(round(float(contact_map.sum(dtype=np.float64)))), 1)
    e_pair = (S_pair - pair_ii - band_pair) / n_pairs

    diff = coords.astype(np.float64)[1:] - coords.astype(np.float64)[:-1]
    bond = np.sqrt((diff * diff).sum(axis=1))
    e_bond = float(((bond - IDEAL_BOND) ** 2).mean())

    total = W_BOND * e_bond + W_CLASH * e_clash + W_PAIR * e_pair
    return np.array([total], dtype=np.float32)


# revision 29
# speedup vs baseline: 17.1460x; 1.2746x over previous
"""Energy refinement kernel for Trainium2 (8 NeuronCores, SPMD row-sharded).

Math notes
----------
reference() computes, for L=4096 coords [L,3] and a 0/1 contact_map [L,L]:
  e_bond  = mean((||c[i+1]-c[i]|| - 6)^2)                       (O(L), host)
  d[i,j]  = ||c_i - c_j|| (+1e-8)
  e_clash = sum_{j>=i+3} relu(3.4-d)^2 / L
  e_pair  = sum_{contact & |i-j|>=3} (d-9)^2 / max(n_contacts,1)
  total   = e_bond + 2*e_clash + 0.5*e_pair

Device strategy (folded symmetry, row-sharded over 8 cores):
  d2 = A @ B^T with A=[c,|c|^2,1], B=[-2c,1,|c|^2]  (K=5 matmul -> PSUM)
  d  = sqrt(max(d2, 0))   (clamp kills fp32 matmul round-off on the diagonal)
  d is symmetric, so each unordered pair is visited once: a 128-row block a
  only processes the 17-block cyclic column span [128a, 128a+2176) — block
  offsets 0..16 of 32.  Offset-0 (within-block) and offset-16 pairs appear
  twice across the whole grid and are halved.  Per 128-row tile the device
  returns per-partition sums of:
    clash = (min(d,3.4)-3.4)^2 = relu(3.4-d)^2, in 3 column sub-ranges
            (diag block / offsets 1..15 / offset-16 block) so the host can
            halve the double-counted slots;
    pair  = ((d-9)*cfold)^2 where cfold = sqrt(w*(c_ij+c_ji)) is built on
            the host (w = 1/2 on the offset-0/16 blocks) — squaring on
            device turns it into w*(c_ij+c_ji)*(d-9)^2.
  The column span and cfold differ per core, so both are shipped as data
  (pre-gathered B columns) keeping the SPMD program identical on all cores.
Host finishing (float64, all O(L) or one O(L^2) pass for cfold):
  subtract the i=j terms and the |i-j|<=2 band (computed with the device's
  own f32 formula), halve the double-counted slots, divide, add bond term.
"""

import numpy as np

L = 4096
NCORES = 8
RPC = L // NCORES          # 512 rows per core
RT = RPC // 128            # 4 row tiles of 128 partitions
SPAN = 17 * 128            # 2176 columns per row tile (block offsets 0..16)
MIN_DIST = 3.4
TARGET_DIST = 9.0
IDEAL_BOND = 6.0
W_BOND, W_CLASH, W_PAIR = 1.0, 2.0, 0.5


def _build_nc(reps=1):
    import concourse.bass as bass
    import concourse.bacc as bacc
    import concourse.mybir as mybir
    import concourse.tile as tile

    f32 = mybir.dt.float32
    AF = mybir.ActivationFunctionType
    ALU = mybir.AluOpType

    # Bacc (not Bass): its compile() runs move_matmul_waits_to_ldweights,
    # required because walrus allows only one sync wait per Matmult.
    nc = bacc.Bacc(None)
    # ab = [at | btfold x4] so a single DMA (one wait semaphore) loads all
    # matmul operands — walrus allows only one sync wait per Matmult.
    ab = nc.declare_dram_parameter("ab", [5, RPC + RT * SPAN], f32, isOutput=False)
    cfold = nc.declare_dram_parameter("cfold", [RPC, SPAN], f32, isOutput=False)
    o_clash = nc.declare_dram_parameter("o_clash", [128, RT * 3], f32, isOutput=True)
    o_pair = nc.declare_dram_parameter("o_pair", [128, RT], f32, isOutput=True)

    HSP = SPAN // 2  # 1088: clamp op width

    with tile.TileContext(nc) as tc:
        with (
            tc.tile_pool(name="const", bufs=1) as constp,
            tc.tile_pool(name="cfp", bufs=3) as cfp,
            tc.tile_pool(name="work", bufs=2) as work,
            tc.tile_pool(name="accp", bufs=1) as accp,
            tc.tile_pool(name="psum", bufs=1, space=bass.MemorySpace.PSUM) as psum,
        ):
            ab_sb = constp.tile([5, RPC + RT * SPAN], f32)
            nc.sync.dma_start(ab_sb[:], ab[:])

            acc_clash = accp.tile([128, RT * 3], f32)
            acc_pair = accp.tile([128, RT], f32)

            for rep in range(reps):
                for it in range(RT):
                    ct = cfp.tile([128, SPAN], f32, tag="ct")
                    nc.sync.dma_start(
                        ct[:], cfold[it * 128 : (it + 1) * 128, :]
                    )
                    lhs = ab_sb[:, it * 128 : (it + 1) * 128]
                    rbase = RPC + it * SPAN
                    ps = psum.tile([128, SPAN], f32, tag="d2")
                    off = 0
                    for n in (512, 512, 512, 512, 128):
                        nc.tensor.matmul(
                            ps[:, off : off + n],
                            lhs,
                            ab_sb[:, rbase + off : rbase + off + n],
                            start=True,
                            stop=True,
                        )
                        off += n

                    # clamp fp32 matmul round-off (diagonal) to 0 before sqrt
                    t_u = work.tile([128, SPAN], f32, tag="t_u")
                    for h in range(2):
                        nc.scalar.activation(
                            t_u[:, h * HSP : (h + 1) * HSP],
                            ps[:, h * HSP : (h + 1) * HSP],
                            AF.Relu,
                        )
                    t_d = work.tile([128, SPAN], f32, tag="t_d")
                    nc.scalar.activation(t_d[:], t_u[:], AF.Sqrt)

                    # clash: relu(3.4-d)^2 summed per column sub-range
                    t_m = work.tile([128, SPAN], f32, tag="t_m")
                    nc.vector.tensor_scalar(
                        t_m[:], t_d[:], MIN_DIST, MIN_DIST, ALU.min, ALU.subtract
                    )
                    t_j0 = work.tile([128, SPAN], f32, tag="junk")
                    for lo, hi, s in ((0, 128, 0), (128, 2048, 1), (2048, SPAN, 2)):
                        nc.scalar.activation(
                            t_j0[:, lo:hi],
                            t_m[:, lo:hi],
                            AF.Square,
                            accum_out=acc_clash[:, it * 3 + s : it * 3 + s + 1],
                        )

                    # pair: ((d-9)*cfold)^2 summed — two fused DVE passes
                    t_x = work.tile([128, SPAN], f32, tag="t_x")
                    nc.vector.scalar_tensor_tensor(
                        t_x[:], t_d[:], TARGET_DIST, ct[:], ALU.subtract, ALU.mult
                    )
                    t_j1 = work.tile([128, SPAN], f32, tag="junk")
                    nc.vector.scalar_tensor_tensor(
                        t_j1[:],
                        t_x[:],
                        1.0,
                        t_x[:],
                        ALU.mult,
                        ALU.mult,
                        accum_out=acc_pair[:, it : it + 1],
                    )

            nc.sync.dma_start(o_clash[:], acc_clash[:])
            nc.sync.dma_start(o_pair[:], acc_pair[:])
    nc.compile()
    return nc


def _augmented(coords):
    """A, B in float32 such that (A @ B.T)[i,j] ~= ||c_i - c_j||^2."""
    c = np.asarray(coords, dtype=np.float32)
    n2 = (c * c).sum(axis=1, dtype=np.float32).astype(np.float32)
    one = np.ones((c.shape[0], 1), dtype=np.float32)
    A = np.concatenate([c, n2[:, None], one], axis=1).astype(np.float32)
    B = np.concatenate([(-2.0 * c).astype(np.float32), one, n2[:, None]], axis=1)
    return A, B.astype(np.float32)


def _emul_offset(A, B, offset):
    """f32-emulated device d-hat for pairs (i, (i+offset) % L)."""
    i = np.arange(L)
    j = (i + offset) % L
    s = np.zeros(L, dtype=np.float32)
    for m in range(5):
        s = (s + A[i, m] * B[j, m]).astype(np.float32)
    return np.sqrt(np.maximum(s, np.float32(0.0)).astype(np.float64)), i, j


def _host_inputs(coords, contact_map):
    A, B = _augmented(coords)
    AT = np.ascontiguousarray(A.T)  # [5, L]
    BT = np.ascontiguousarray(B.T)
    w = np.ones(SPAN, dtype=np.float32)
    w[:128] = 0.5
    w[2048:] = 0.5
    in_maps = []
    for r in range(NCORES):
        parts = [AT[:, r * RPC : (r + 1) * RPC]]
        cf_r = np.empty((RPC, SPAN), dtype=np.float32)
        for it in range(RT):
            a = r * RT + it
            i0 = a * 128
            cols = np.arange(i0, i0 + SPAN) % L
            parts.append(BT[:, cols])
            cf = (
                contact_map[i0 : i0 + 128][:, cols]
                + contact_map[cols][:, i0 : i0 + 128].T
            ) * w
            np.sqrt(cf, out=cf)
            cf_r[it * 128 : (it + 1) * 128] = cf
        in_maps.append(
            {
                "ab": np.ascontiguousarray(np.concatenate(parts, axis=1)),
                "cfold": cf_r,
            }
        )
    return A, B, in_maps


_CACHE = {}


def kernel(coords, contact_map):
    from concourse.bass_utils import run_bass_kernel_spmd

    coords = np.asarray(coords, dtype=np.float32)
    # reference semantics: a pair is a contact iff contact_map > 0.5
    contact_map = np.ascontiguousarray(
        (np.asarray(contact_map) > 0.5).astype(np.float32)
    )
    A, B, in_maps = _host_inputs(coords, contact_map)

    if "nc" not in _CACHE:
        _CACHE["nc"] = _build_nc()
    res = run_bass_kernel_spmd(_CACHE["nc"], in_maps, list(range(NCORES))).results

    S_main = S_diag = S_s16 = S_pair = 0.0
    for r in range(NCORES):
        oc = res[r]["o_clash"].astype(np.float64)
        S_diag += float(oc[:, 0::3].sum())
        S_main += float(oc[:, 1::3].sum())
        S_s16 += float(oc[:, 2::3].sum())
        S_pair += float(res[r]["o_pair"].astype(np.float64).sum())

    # host corrections, all O(L), emulating the device's own f32 formula
    dh0, ii, _ = _emul_offset(A, B, 0)
    cl0 = np.minimum(dh0, MIN_DIST) - MIN_DIST
    emul_ii = float((cl0 * cl0).sum())
    pair_ii = float(
        (contact_map[ii, ii].astype(np.float64) * (dh0 - TARGET_DIST) ** 2).sum()
    )
    band_clash = 0.0
    band_pair = 0.0
    for s_off in (1, 2):
        dh, i, j = _emul_offset(A, B, s_off)
        keep = i + s_off < L  # wrapped pairs have linear sep >= 3: not banded
        cl = np.minimum(dh[keep], MIN_DIST) - MIN_DIST
        band_clash += float((cl * cl).sum())
        cw = (
            contact_map[i[keep], j[keep]].astype(np.float64)
            + contact_map[j[keep], i[keep]].astype(np.float64)
        )
        band_pair += float((cw * (dh[keep] - TARGET_DIST) ** 2).sum())

    S_clash_u = S_main + (S_diag - emul_ii) / 2.0 + S_s16 / 2.0
    e_clash = (S_clash_u - band_clash) / L

    n_pairs = max(int(round(float(contact_map.sum(dtype=np.float64)))), 1)
    e_pair = (S_pair - pair_ii - band_pair) / n_pairs

    diff = coords.astype(np.float64)[1:] - coords.astype(np.float64)[:-1]
    bond = np.sqrt((diff * diff).sum(axis=1))
    e_bond = float(((bond - IDEAL_BOND) ** 2).mean())

    total = W_BOND * e_bond + W_CLASH * e_clash + W_PAIR * e_pair
    return np.array([total], dtype=np.float32)
